# revision 1
# baseline (speedup 1.0000x reference)
"""AdaptiveMambaBlock on 8 TRN2 NeuronCores (Bass/Tile, SPMD).

Sharding: 8-way over tokens. Core c handles batch c//4, token range
[(c%4)*1024, +1024). Each core runs the full pipeline for its tokens in a
feature-major ("transposed") layout; host pre-packs weights into the exact
tile layouts the device wants and adds the x residual after gathering.

Scan: s^c_t = A s^c_{t-1} + v^c_t with v computed directly from the
normalized input (v = xhat @ (Bm@W1*gamma).T), A diagonalized on the host so
the device runs a per-partition linear recurrence (tensor_tensor_scan).
Cross-core state is stitched with a 64B-per-core AllGather of the local scan
tails.
"""

import os
import numpy as np
import ml_dtypes

import concourse.bass as bass
import concourse.tile as tile
from concourse import bacc
from concourse import mybir
from concourse.bass_utils import run_bass_kernel_spmd
from concourse.masks import make_identity

F32 = mybir.dt.float32
F32R = mybir.dt.float32r
BF16 = mybir.dt.bfloat16

D_MODEL, D_STATE, D_CONV = 1024, 16, 4
D_INNER = 2048
B, L = 2, 4096
LN_EPS = 1e-5
N_CORES = 8
TLOC = 1024           # tokens per core
KT = D_MODEL // 128   # 8 K tiles over d_model
FT = 2 * D_INNER // 128  # 32 feature tiles (x-part 0..15, z 16..31)
CT = D_INNER // 128   # 16 channel tiles
MT = D_MODEL // 128   # 8 output (d_model) tiles
NCH = TLOC // 512     # 2 token chunks of 512

_NC_CACHE = None
LAST_RESULT = None


def r32(ap):
    return ap.bitcast(F32R)


def build_graph():
    nc = bacc.Bacc(num_devices=N_CORES)

    xT = nc.declare_dram_parameter("xT", [D_MODEL, TLOC], F32R, isOutput=False)
    wint = nc.declare_dram_parameter("wint", [FT, 128, KT, 128], F32R, isOutput=False)
    wvbt = nc.declare_dram_parameter("wvbt", [128, KT, D_STATE], F32R, isOutput=False)
    cmt = nc.declare_dram_parameter("cmt", [D_STATE, D_INNER], F32R, isOutput=False)
    wot = nc.declare_dram_parameter("wot", [MT, 128, CT, 128], BF16, isOutput=False)
    convw = nc.declare_dram_parameter("convw", [128, CT, D_CONV], F32, isOutput=False)
    convb = nc.declare_dram_parameter("convb", [128, CT], F32, isOutput=False)
    biasz = nc.declare_dram_parameter("biasz", [128, CT], F32, isOutput=False)
    uhalo = nc.declare_dram_parameter("uhalo", [128, CT, 3], BF16, isOutput=False)
    vadd = nc.declare_dram_parameter("vadd", [D_STATE, TLOC], F32, isOutput=False)
    decay = nc.declare_dram_parameter("decay", [D_STATE, 1], F32, isOutput=False)
    gct = nc.declare_dram_parameter("gct", [D_STATE, N_CORES], F32, isOutput=False)
    res = nc.declare_dram_parameter("res", [D_MODEL, TLOC], F32, isOutput=True)

    with tile.TileContext(nc) as tc:
        with (
            tc.tile_pool(name="sb", bufs=1) as sb,
            tc.tile_pool(name="sb2", bufs=2) as sb2,
            tc.tile_pool(name="ps", bufs=6, space="PSUM") as ps,
            tc.tile_pool(name="pss", bufs=2, space="PSUM") as pss,
            tc.tile_pool(name="dr", bufs=1, space="DRAM") as dr,
        ):
            with nc.allow_low_precision(reason="f32r matmul pipeline"):
                _emit(nc, tc, sb, sb2, ps, pss, dr, locals())
    nc.compile()
    return nc


def _emit(nc, tc, sb, sb2, ps, pss, dr, t):
    xT, wint, wvbt, cmt, wot = t["xT"], t["wint"], t["wvbt"], t["cmt"], t["wot"]
    convw, convb, biasz = t["convw"], t["convb"], t["biasz"]
    uhalo, vadd, decay, gct, res = t["uhalo"], t["vadd"], t["decay"], t["gct"], t["res"]

    # ---- x load first (per half, per K tile): gates the stats chain.
    # The cost model serializes DMA, so the first two in_proj weight tiles
    # are prefetched between the two x halves — otherwise wt[f0] lands
    # after xhat and gates the first matmul.
    x_sb = sb.tile([128, KT, TLOC], F32R)
    xTr = xT.rearrange("(ko ki) t -> ki ko t", ki=128)
    wt_pre = {}
    for ko in range(KT):
        nc.sync.dma_start(out=x_sb[:, ko, 0:512], in_=xTr[:, ko, 0:512])
    for f in range(2):
        w = sb2.tile([128, KT, 128], F32R, name="wt", bufs=4)
        nc.sync.dma_start(out=w, in_=wint[f])
        wt_pre[f] = w
    for ko in range(KT):
        nc.sync.dma_start(out=x_sb[:, ko, 512:1024], in_=xTr[:, ko, 512:1024])

    # ---- constants / small loads -------------------------------------
    # memset can't target f32r; memset f32 then DVE-copy (a rounding producer)
    ones_k0 = sb.tile([128, 1], F32)
    nc.vector.memset(ones_k0, 1.0 / D_MODEL)
    ones_k = sb.tile([128, 1], F32R)         # 1/D_MODEL, stats lhsT
    nc.vector.tensor_copy(ones_k, ones_k0)
    ones_b0 = sb.tile([1, 128], F32)
    nc.vector.memset(ones_b0, 1.0)
    ones_b = sb.tile([1, 128], F32R)         # broadcast lhsT (K=1, f32r)
    nc.vector.tensor_copy(ones_b, ones_b0)
    rr = sb.tile([1, 1024], F32R)            # f32r staging rows for bcast rhs
    eps_t = sb.tile([1, 1], F32)
    nc.vector.memset(eps_t, LN_EPS)
    ident = sb.tile([128, 128], BF16)
    make_identity(nc, ident)
    convw_sb = sb.tile([128, CT, D_CONV], F32)
    nc.sync.dma_start(out=convw_sb, in_=t["convw"][:])
    convb_sb = sb.tile([128, CT], F32)
    nc.sync.dma_start(out=convb_sb, in_=convb[:])
    biasz_sb = sb.tile([128, CT], F32)
    nc.sync.dma_start(out=biasz_sb, in_=biasz[:])
    # elementwise engines are partition-locked: every op's in/out must sit on
    # the same partitions, so all small tensors live at base partition 0 and
    # are packed along the free dimension instead.
    # slot A: mu then (dead after mur); slot B: msq->var->std->mur;
    # slot C: mu^2 -> r.   states: v | decay_t | vadd-then-l (l overwrites
    # vadd after v is finalized).
    rows = sb.tile([1, 3 * 512], F32)  # per-half: mu | var | scratch
    states = sb.tile([D_STATE, 2 * TLOC], F32)
    s_sb = sb.tile([D_STATE, TLOC], F32R)       # scan #2 output (sC rhs)
    vadd_sb = states[:, TLOC : 2 * TLOC]
    nc.sync.dma_start(out=vadd_sb, in_=vadd[:])
    decay_c = sb.tile([D_STATE, 1], F32)
    nc.sync.dma_start(out=decay_c, in_=decay[:])
    gct_sb = sb.tile([D_STATE, N_CORES], F32)
    nc.sync.dma_start(out=gct_sb, in_=gct[:])

    # ---- layernorm stats (per 512-token half, pipelined) -----------
    # mu/msq rows via ones-matmul; square on ACT; var/recip chain per half so
    # normalize (and in_proj) can start on half 0 while half 1 is in flight.
    mu_row = rows[:, 0:512]          # all three slots are per-half scratch
    var_row = rows[:, 512:1024]
    mu2_row = rows[:, 1024:1536]
    rb_sb = sb.tile([128, TLOC], F32)
    murb_sb = sb.tile([128, TLOC], F32)

    sq_half = []
    for n in range(NCH):
        cs = slice(n * 512, (n + 1) * 512)
        mu_ps = pss.tile([1, 512], F32, tag="sm", name=f"mu_ps{n}")
        sq_ps = pss.tile([1, 512], F32, tag="sm", name=f"sq_ps{n}")
        for ko in range(KT):
            sq_scr = sb2.tile([128, 512], F32R, name="sq_scr", bufs=2)
            nc.scalar.square(sq_scr, x_sb[:, ko, cs])
            nc.tensor.matmul(mu_ps, ones_k, x_sb[:, ko, cs],
                             start=(ko == 0), stop=(ko == KT - 1))
            nc.tensor.matmul(sq_ps, ones_k, sq_scr,
                             start=(ko == 0), stop=(ko == KT - 1))
        # read stat rows straight from PSUM: mu^2 and msq-mu^2 consume the
        # psum tiles directly, trimming two copies off the critical chain
        nc.vector.tensor_copy(mu_row, mu_ps)
        nc.vector.tensor_mul(mu2_row, mu_row, mu_ps)
        nc.vector.tensor_sub(var_row, sq_ps, mu2_row)
        # fused rsqrt on ACT (Abs_reciprocal_sqrt; var+eps > 0), f32r out
        rr_r, rr_mur = rr[:, 0:512], rr[:, 512:1024]
        nc.scalar.activation(rr_r, var_row,
                             mybir.ActivationFunctionType.Abs_reciprocal_sqrt,
                             bias=eps_t, scale=1.0)
        nc.vector.tensor_mul(rr_mur, mu_row, rr_r)   # f32r out

        # broadcast r, mur to 128 partitions via K=1 f32r matmul
        for srow, dst in ((rr_r, rb_sb), (rr_mur, murb_sb)):
            b_ps = pss.tile([128, 512], F32, tag="sm", name="b_ps")
            nc.tensor.matmul(b_ps, ones_b, srow, start=True, stop=True)
            nc.vector.tensor_copy(dst[:, cs], b_ps)

        # normalize this half in place: xhat = x*rb - murb
        for ko in range(KT):
            nc.vector.tensor_mul(x_sb[:, ko, cs], x_sb[:, ko, cs], rb_sb[:, cs])
            nc.vector.tensor_sub(x_sb[:, ko, cs], x_sb[:, ko, cs], murb_sb[:, cs])

    # ---- v + scan #1 + collective -----------------------------------
    wvb_sb = sb.tile([128, KT, D_STATE], F32R)
    nc.sync.dma_start(out=wvb_sb, in_=wvbt[:])
    v_sb = states[:, 0:TLOC]
    for n in range(NCH):
        cs = slice(n * 512, (n + 1) * 512)
        v_ps = pss.tile([D_STATE, 512], F32, tag="sm", name="v_ps")
        for ko in range(KT):
            nc.tensor.matmul(v_ps, wvb_sb[:, ko, :], x_sb[:, ko, cs],
                             start=(ko == 0), stop=(ko == KT - 1))
        nc.vector.tensor_add(v_sb[:, cs], v_ps, vadd_sb[:, cs])

    # broadcast-AP view of the decay column (free stride 0) for the scans
    decay_t = decay_c.broadcast_to([D_STATE, TLOC])

    l_sb = vadd_sb  # vadd is dead once v is finalized
    nc.vector.tensor_tensor_scan(l_sb, decay_t, v_sb, 0.0,
                                 mybir.AluOpType.mult, mybir.AluOpType.add)

    cc_in = dr.tile([D_STATE, 1], F32)
    cc_out = dr.tile([D_STATE * N_CORES, 1], F32, addr_space="Shared")
    nc.sync.dma_start(out=cc_in[:], in_=l_sb[:, TLOC - 1 : TLOC])
    nc.gpsimd.collective_compute(
        "AllGather", mybir.AluOpType.bypass,
        replica_groups=[list(range(N_CORES))],
        ins=[cc_in[:]], outs=[cc_out[:]],
    )
    lam_all = sb.tile([D_STATE, N_CORES], F32)
    nc.sync.dma_start(out=lam_all,
                      in_=cc_out.rearrange("(j d) one -> d (j one)", d=D_STATE))
    sig_scr = sb.tile([D_STATE, N_CORES], F32)
    sigma = sb.tile([D_STATE, 1], F32)
    nc.vector.scalar_tensor_tensor(
        out=sig_scr, in0=lam_all, scalar=1.0, in1=gct_sb,
        op0=mybir.AluOpType.mult, op1=mybir.AluOpType.mult, accum_out=sigma)
    nc.vector.tensor_tensor_scan(s_sb, decay_t, v_sb, sigma,
                                 mybir.AluOpType.mult, mybir.AluOpType.add)

    # ---- in_proj ----------------------------------------------------
    u_sb = sb.tile([128, CT, TLOC + 3], BF16)      # halo(3) + tokens
    nc.sync.dma_start(out=u_sb[:, :, 0:3], in_=uhalo[:])
    sigz_sb = sb.tile([128, CT, TLOC], BF16)
    for f in range(FT):
        if f in wt_pre:
            wt = wt_pre[f]
        else:
            wt = sb2.tile([128, KT, 128], F32R, name="wt", bufs=4)
            nc.sync.dma_start(out=wt, in_=wint[f])
        for n in range(NCH):
            cs = slice(n * 512, (n + 1) * 512)
            p_t = ps.tile([128, 512], F32, tag="mm", name=f"ip{f}_{n}")
            for ko in range(KT):
                nc.tensor.matmul(p_t, wt[:, ko, :], x_sb[:, ko, cs],
                                 start=(ko == 0), stop=(ko == KT - 1))
            if f < CT:  # x-part -> u (bias folded into conv bias on host)
                nc.scalar.copy(
                    out=u_sb[:, f, 3 + n * 512 : 3 + (n + 1) * 512],
                    in_=p_t)
            else:       # z -> sigmoid(z + bias)
                c = f - CT
                nc.scalar.activation(
                    out=sigz_sb[:, c, n * 512 : (n + 1) * 512],
                    in_=p_t, func=mybir.ActivationFunctionType.Sigmoid,
                    bias=biasz_sb[:, c : c + 1], scale=1.0)

    # ---- conv (DVE MACs) + silu + sC + gating, per channel tile ----
    cmt_sb = sb.tile([D_STATE, D_INNER], F32R)
    nc.sync.dma_start(out=cmt_sb, in_=cmt[:])
    y_sb = sb.tile([128, CT, TLOC], BF16)
    for c in range(CT):
        # full-width (128,1024) MACs: fewer DVE ops, same math
        acc = sb2.tile([128, TLOC], F32, name="cacc", bufs=2)
        nc.vector.tensor_scalar_mul(
            out=acc, in0=u_sb[:, c, 0:TLOC], scalar1=convw_sb[:, c, 0:1])
        for tap in range(1, D_CONV):
            nc.vector.scalar_tensor_tensor(
                out=acc, in0=u_sb[:, c, tap : tap + TLOC],
                scalar=convw_sb[:, c, tap : tap + 1], in1=acc,
                op0=mybir.AluOpType.mult, op1=mybir.AluOpType.add)
        # silu(conv + b') full width, back into the u slot
        nc.scalar.activation(
            out=u_sb[:, c, 3 : 3 + TLOC], in_=acc,
            func=mybir.ActivationFunctionType.Silu,
            bias=convb_sb[:, c : c + 1], scale=1.0)
        for n in range(NCH):
            cs = slice(n * 512, (n + 1) * 512)
            sc_ps = ps.tile([128, 512], F32, tag="mm", name=f"sc{c}_{n}")
            nc.tensor.matmul(sc_ps, cmt_sb[:, c * 128 : (c + 1) * 128],
                             s_sb[:, cs], start=True, stop=False)
            # accumulate silu(u_conv) into the same PSUM via identity matmul
            nc.tensor.matmul(sc_ps, ident,
                             u_sb[:, c, 3 + n * 512 : 3 + (n + 1) * 512],
                             start=False, stop=True)
            nc.vector.tensor_mul(y_sb[:, c, cs], sc_ps, sigz_sb[:, c, cs])

    # ---- out_proj: out[M=dm, N=tok] = sum_c wo[c].T @ y[c] ----------
    for m in range(MT):
        wo = sb2.tile([128, CT, 128], BF16, name="wo", bufs=2)
        nc.sync.dma_start(out=wo, in_=wot[m])
        for n in range(NCH):
            cs = slice(n * 512, (n + 1) * 512)
            o_ps = ps.tile([128, 512], F32, tag="mm", name=f"op{m}_{n}")
            for c in range(CT):
                nc.tensor.matmul(o_ps, wo[:, c, :], y_sb[:, c, cs],
                                 start=(c == 0), stop=(c == CT - 1))
            r_sb = sb2.tile([128, 512], F32, name="r_sb", bufs=1)
            nc.scalar.copy(r_sb, o_ps)
            nc.sync.dma_start(out=res[m * 128 : (m + 1) * 128, cs], in_=r_sb)


# ---------------------------------------------------------------------
# host side
# ---------------------------------------------------------------------

def _standardize(x):
    mu = x.mean(-1, keepdims=True)
    var = ((x - mu) ** 2).mean(-1, keepdims=True)
    return ((x - mu) / np.sqrt(var + LN_EPS)).astype(np.float32)


def host_prepare(inputs):
    x = np.ascontiguousarray(np.asarray(inputs["x"], np.float32))
    g = np.asarray(inputs["ln_gamma"], np.float32)
    beta = np.asarray(inputs["ln_beta"], np.float32)
    W_in = np.asarray(inputs["W_in"], np.float32)
    conv_w = np.asarray(inputs["conv_w"], np.float32)[:, 0, :]
    conv_b = np.asarray(inputs["conv_b"], np.float32)
    W_out = np.asarray(inputs["W_out"], np.float32)
    A = np.asarray(inputs["A"], np.float32)
    Bm = np.asarray(inputs["Bm"], np.float32)
    Cm = np.asarray(inputs["Cm"], np.float32)

    Wg = W_in * g[None, :]
    b_in = W_in @ beta
    bias_u = b_in[:D_INNER]
    bias_z = b_in[D_INNER:]
    W1g = Wg[:D_INNER]

    Wvb0 = (Bm @ W_in[:D_INNER]) * g[None, :]
    bias_v0 = Bm @ W_in[:D_INNER] @ beta

    fallback = False
    lamc, V = np.linalg.eig(A.astype(np.float64))
    if np.abs(lamc.imag).max() > 1e-9 or np.linalg.cond(V) > 1e3:
        fallback = True
    if fallback:
        lam = np.zeros(D_STATE, np.float32)
        Wvb = np.zeros_like(Wvb0)
        Cmt = Cm.astype(np.float32)
        # full scan on host
        xn = _standardize(x.reshape(-1, D_MODEL)).reshape(x.shape) * g + beta
        v = xn.astype(np.float32) @ (Bm @ W_in[:D_INNER]).T
        sT = np.zeros((B, L, D_STATE), np.float32)
        for b_ in range(B):
            cur = np.zeros(D_STATE, np.float64)
            Ad = A.astype(np.float64)
            for tt in range(L):
                cur = Ad @ cur + v[b_, tt]
                sT[b_, tt] = cur
        # keep device inputs finite even when the recurrence diverges
        # (the reference diverges identically for such an A)
        sT = np.nan_to_num(sT, posinf=3e38, neginf=-3e38)
    else:
        lam = lamc.real
        Vr = V.real
        Vi = np.linalg.inv(Vr)
        Wvb = (Vi @ Wvb0).astype(np.float32)
        bias_vt = (Vi @ bias_v0).astype(np.float32)
        Cmt = (Vr.T @ Cm).astype(np.float32)

    # packed weight layouts
    wint = np.ascontiguousarray(
        Wg.reshape(FT, 128, KT, 128).transpose(0, 3, 2, 1))
    wvbt = np.ascontiguousarray(
        Wvb.reshape(D_STATE, KT, 128).transpose(2, 1, 0)) if not fallback \
        else np.zeros((128, KT, D_STATE), np.float32)
    wot = np.ascontiguousarray(
        W_out.reshape(MT, 128, CT, 128).transpose(0, 3, 2, 1)
    ).astype(ml_dtypes.bfloat16)
    convw_p = np.ascontiguousarray(conv_w.reshape(CT, 128, D_CONV).transpose(1, 0, 2))
    # u is produced biasless on device; fold bias_u through the conv taps
    convb_f = conv_b + bias_u * conv_w.sum(axis=1)
    convb_p = np.ascontiguousarray(convb_f.reshape(CT, 128).T)
    biasz_p = np.ascontiguousarray(bias_z.reshape(CT, 128).T)
    decay_p = lam.astype(np.float32).reshape(D_STATE, 1)

    in_maps = []
    for c in range(N_CORES):
        b_, k = c // 4, c % 4
        xs = x[b_, k * TLOC : (k + 1) * TLOC]            # (1024, 1024)
        xTc = np.ascontiguousarray(xs.T)

        if k == 0:
            uh = np.zeros((D_INNER, 3), np.float32)
        else:
            xh = x[b_, k * TLOC - 3 : k * TLOC]
            uh = (_standardize(xh) @ W1g.T).T  # biasless; bias folded into conv_b
        uh_p = np.ascontiguousarray(
            uh.reshape(CT, 128, 3).transpose(1, 0, 2)).astype(ml_dtypes.bfloat16)

        if fallback:
            va = np.ascontiguousarray(sT[b_, k * TLOC : (k + 1) * TLOC].T)
            G = np.zeros((N_CORES, D_STATE), np.float32)
        else:
            va = np.broadcast_to(bias_vt[:, None], (D_STATE, TLOC)).copy()
            G = np.zeros((N_CORES, D_STATE), np.float32)
            for j in range(N_CORES):
                bj, kj = j // 4, j % 4
                if bj == b_ and kj < k:
                    G[j] = lam ** (TLOC * (k - kj))
        in_maps.append(dict(
            xT=xTc, wint=wint, wvbt=wvbt, cmt=Cmt.astype(np.float32),
            wot=wot, convw=convw_p, convb=convb_p,
            biasz=biasz_p, uhalo=uh_p, vadd=va.astype(np.float32),
            decay=decay_p, gct=np.ascontiguousarray(G.T),
        ))
    return in_maps, x


def get_nc():
    global _NC_CACHE
    if _NC_CACHE is None:
        _NC_CACHE = build_graph()
    return _NC_CACHE


def kernel(**inputs):
    global LAST_RESULT
    nc = get_nc()
    in_maps, x = host_prepare(inputs)
    trace = bool(os.environ.get("BASS_TRACE"))
    r = run_bass_kernel_spmd(nc, in_maps, core_ids=list(range(N_CORES)),
                             trace=trace)
    LAST_RESULT = r
    out = np.empty((B, L, D_MODEL), np.float32)
    for c in range(N_CORES):
        b_, k = c // 4, c % 4
        resT = r.results[c]["res"]                      # (d_model, tok)
        out[b_, k * TLOC : (k + 1) * TLOC] = (
            x[b_, k * TLOC : (k + 1) * TLOC] + resT.T)
    return out



# revision 4
# speedup vs baseline: 1.3381x; 1.3381x over previous
"""AdaptiveMambaBlock on 8 TRN2 NeuronCores (Bass/Tile, SPMD) — fp8 DoubleRow.

Sharding: 8-way over tokens. Core c handles batch c//4, token range
[(c%4)*1024, +1024). Feature-major layout; host pre-packs weights.

v2: the two big GEMMs (in_proj, out_proj) run as fp8e4m3 DoubleRow matmuls
(K=256 per instruction at 0.5 cyc/row). in_proj quantizes both operands
(per-output-row weight scales folded into the PSUM-drain copies); out_proj
splits the weight into hi+lo e4m3 parts sharing one row scale, paired in a
single DR instruction against a stride-0-broadcast y8 k-tile, which keeps
out_proj error at the y-quantization level. The d_state recurrence (v,
scan, collective stitch, sC) stays in bf16/f32 since the cumsum amplifies
quantization error. Elementwise work: x/xhat/u/σ(z) in bf16 (DVE 2x/4x
modes), drains on Act, xhat→fp8 quant split across Pool/DVE.

Scan: s^c_t = A s^c_{t-1} + v^c_t with v from the normalized input
(v = xhat @ (Bm@W1*gamma).T), A diagonalized on the host; cross-core state
stitched with a 64B-per-core AllGather of the local scan tails.
"""

import os
import numpy as np
import ml_dtypes

import concourse.bass as bass
import concourse.tile as tile
from concourse import bacc
from concourse import mybir
from concourse.bass_utils import run_bass_kernel_spmd
from concourse.masks import make_identity

F32 = mybir.dt.float32
F32R = mybir.dt.float32r
BF16 = mybir.dt.bfloat16
FP8 = mybir.dt.float8e4
E4 = ml_dtypes.float8_e4m3
BF = ml_dtypes.bfloat16
DR = mybir.MatmulPerfMode.DoubleRow

D_MODEL, D_STATE, D_CONV = 1024, 16, 4
D_INNER = 2048
B, L = 2, 4096
LN_EPS = 1e-5
N_CORES = 8
TLOC = 1024              # tokens per core
KT = D_MODEL // 128      # 8 K tiles over d_model
DRK = KT // 2            # 4 DoubleRow K pairs for in_proj
FT = 2 * D_INNER // 128  # 32 feature tiles (x-part 0..15, z 16..31)
CT = D_INNER // 128      # 16 channel tiles
MT = D_MODEL // 128      # 8 output (d_model) tiles
OKT = D_INNER // 128     # 16 out_proj K tiles
NCH = TLOC // 512        # 2 token chunks of 512

_NC_CACHE = None
LAST_RESULT = None


def build_graph():
    nc = bacc.Bacc(num_devices=N_CORES)

    xT = nc.declare_dram_parameter("xT", [D_MODEL, TLOC], BF16, isOutput=False)
    win8 = nc.declare_dram_parameter("win8", [FT, 128, DRK, 2, 128], FP8, isOutput=False)
    wsc = nc.declare_dram_parameter("wsc", [128, FT], F32, isOutput=False)
    wvbt = nc.declare_dram_parameter("wvbt", [128, KT, D_STATE], BF16, isOutput=False)
    cmt = nc.declare_dram_parameter("cmt", [D_STATE, D_INNER], F32R, isOutput=False)
    wot8 = nc.declare_dram_parameter("wot8", [MT, 128, OKT, 2, 128], FP8, isOutput=False)
    wosc = nc.declare_dram_parameter("wosc", [128, MT], F32, isOutput=False)
    convw = nc.declare_dram_parameter("convw", [128, CT, D_CONV], F32, isOutput=False)
    convb = nc.declare_dram_parameter("convb", [128, CT], F32, isOutput=False)
    biasz = nc.declare_dram_parameter("biasz", [128, CT], F32, isOutput=False)
    uhalo = nc.declare_dram_parameter("uhalo", [128, CT, 3], BF16, isOutput=False)
    vadd = nc.declare_dram_parameter("vadd", [D_STATE, TLOC], F32, isOutput=False)
    decay = nc.declare_dram_parameter("decay", [D_STATE, 1], F32, isOutput=False)
    gct = nc.declare_dram_parameter("gct", [D_STATE, N_CORES], F32, isOutput=False)
    res = nc.declare_dram_parameter("res", [D_MODEL, TLOC], BF16, isOutput=True)

    with tile.TileContext(nc) as tc:
        with (
            tc.tile_pool(name="sb", bufs=1) as sb,
            tc.tile_pool(name="sb2", bufs=2) as sb2,
            tc.tile_pool(name="ps", bufs=2, space="PSUM") as ps,
            tc.tile_pool(name="pss", bufs=2, space="PSUM") as pss,
            tc.tile_pool(name="dr", bufs=1, space="DRAM") as drm,
        ):
            with nc.allow_low_precision(reason="fp8/bf16 matmul pipeline"):
                _emit(nc, tc, sb, sb2, ps, pss, drm, locals())
    nc.compile()
    return nc


def _emit(nc, tc, sb, sb2, ps, pss, drm, t):
    xT, win8, wsc, wvbt, cmt, wot8, wosc = (
        t["xT"], t["win8"], t["wsc"], t["wvbt"], t["cmt"], t["wot8"], t["wosc"])
    convw, convb, biasz = t["convw"], t["convb"], t["biasz"]
    uhalo, vadd, decay, gct, res = t["uhalo"], t["vadd"], t["decay"], t["gct"], t["res"]

    AF = mybir.ActivationFunctionType

    # ---- x load first (per half, per K tile): gates the stats chain.
    x_sb = sb.tile([128, KT, TLOC], BF16)
    xTr = xT.rearrange("(ko ki) t -> ki ko t", ki=128)
    win_pre = {}
    for ko in range(KT):
        nc.sync.dma_start(out=x_sb[:, ko, 0:512], in_=xTr[:, ko, 0:512])
    # prefetch first in_proj weight tiles between the two x halves
    f_order = []
    for c in range(CT):
        f_order += [CT + c, c]          # z(c) first, then u(c)
    for f in f_order[:2]:
        w = sb2.tile([128, DRK, 2, 128], FP8, name="win", bufs=4)
        nc.sync.dma_start(out=w, in_=win8[f])
        win_pre[f] = w
    for ko in range(KT):
        nc.sync.dma_start(out=x_sb[:, ko, 512:1024], in_=xTr[:, ko, 512:1024])

    # ---- constants / small loads -------------------------------------
    ones_k0 = sb.tile([128, 1], F32)
    nc.vector.memset(ones_k0, 1.0 / D_MODEL)
    ones_k = sb.tile([128, 1], BF16)         # 1/D_MODEL, stats lhsT
    nc.vector.tensor_copy(ones_k, ones_k0)
    ones_b0 = sb.tile([1, 128], F32)
    nc.vector.memset(ones_b0, 1.0)
    ones_b = sb.tile([1, 128], BF16)         # broadcast lhsT (K=1)
    nc.vector.tensor_copy(ones_b, ones_b0)
    rr = sb.tile([1, 1024], BF16)            # bf16 staging rows for bcast rhs
    eps_t = sb.tile([1, 1], F32)
    nc.vector.memset(eps_t, LN_EPS)
    ident = sb.tile([128, 128], BF16)
    make_identity(nc, ident)
    convw_sb = sb.tile([128, CT, D_CONV], F32)
    nc.sync.dma_start(out=convw_sb, in_=convw[:])
    convb_sb = sb.tile([128, CT], F32)
    nc.sync.dma_start(out=convb_sb, in_=convb[:])
    biasz_sb = sb.tile([128, CT], F32)
    nc.sync.dma_start(out=biasz_sb, in_=biasz[:])
    wsc_sb = sb.tile([128, FT], F32)
    nc.sync.dma_start(out=wsc_sb, in_=wsc[:])
    wosc_sb = sb.tile([128, MT], F32)
    nc.sync.dma_start(out=wosc_sb, in_=wosc[:])

    rows = sb.tile([1, 3 * 512], F32)        # per-half: mu | var | scratch
    states = sb.tile([D_STATE, 2 * TLOC], F32)
    s_sb = sb.tile([D_STATE, TLOC], F32R)    # scan #2 output (sC rhs)
    vadd_sb = states[:, TLOC : 2 * TLOC]
    nc.sync.dma_start(out=vadd_sb, in_=vadd[:])
    decay_c = sb.tile([D_STATE, 1], F32)
    nc.sync.dma_start(out=decay_c, in_=decay[:])
    gct_sb = sb.tile([D_STATE, N_CORES], F32)
    nc.sync.dma_start(out=gct_sb, in_=gct[:])

    x8 = sb.tile([128, DRK, 2, TLOC], FP8)   # quantized xhat, DR layout
    rb_sb = sb.tile([128, TLOC], BF16)
    murb_sb = sb.tile([128, TLOC], BF16)

    # ---- layernorm stats (per 512-token half, pipelined) -------------
    mu_row = rows[:, 0:512]
    var_row = rows[:, 512:1024]
    mu2_row = rows[:, 1024:1536]

    for n in range(NCH):
        cs = slice(n * 512, (n + 1) * 512)
        mu_ps = pss.tile([1, 512], F32, tag="sm", name=f"mu_ps{n}")
        sq_ps = pss.tile([1, 512], F32, tag="sm", name=f"sq_ps{n}")
        for ko in range(KT):
            sq_scr = sb2.tile([128, 512], BF16, name="sq_scr", bufs=2)
            nc.vector.tensor_mul(sq_scr, x_sb[:, ko, cs], x_sb[:, ko, cs])
            nc.tensor.matmul(mu_ps, ones_k, x_sb[:, ko, cs],
                             start=(ko == 0), stop=(ko == KT - 1))
            nc.tensor.matmul(sq_ps, ones_k, sq_scr,
                             start=(ko == 0), stop=(ko == KT - 1))
        nc.vector.tensor_copy(mu_row, mu_ps)
        nc.vector.tensor_mul(mu2_row, mu_row, mu_ps)
        nc.vector.tensor_sub(var_row, sq_ps, mu2_row)
        rr_r, rr_mur = rr[:, 0:512], rr[:, 512:1024]
        nc.scalar.activation(rr_r, var_row, AF.Abs_reciprocal_sqrt,
                             bias=eps_t, scale=1.0)
        nc.vector.tensor_mul(rr_mur, mu_row, rr_r)

        for srow, dst in ((rr_r, rb_sb), (rr_mur, murb_sb)):
            b_ps = pss.tile([128, 512], F32, tag="sm", name="b_ps")
            nc.tensor.matmul(b_ps, ones_b, srow, start=True, stop=True)
            nc.vector.tensor_copy(dst[:, cs], b_ps)

        # normalize this half in place: xhat = x*rb - murb (bf16, DVE 2x)
        for ko in range(KT):
            nc.vector.tensor_mul(x_sb[:, ko, cs], x_sb[:, ko, cs], rb_sb[:, cs])
            nc.vector.tensor_sub(x_sb[:, ko, cs], x_sb[:, ko, cs], murb_sb[:, cs])
        # quantize to fp8 (DR rhs layout); Pool for half 0, Act for half 1
        for ko in range(KT):
            dst = x8[:, ko // 2, ko % 2, cs]
            if n == 0:
                nc.gpsimd.tensor_copy(dst, x_sb[:, ko, cs])
            else:
                nc.scalar.copy(dst, x_sb[:, ko, cs])

    # ---- v + scan #1 + collective ------------------------------------
    wvb_sb = sb.tile([128, KT, D_STATE], BF16)
    nc.sync.dma_start(out=wvb_sb, in_=wvbt[:])
    v_sb = states[:, 0:TLOC]
    for n in range(NCH):
        cs = slice(n * 512, (n + 1) * 512)
        v_ps = pss.tile([D_STATE, 512], F32, tag="sm", name="v_ps")
        for ko in range(KT):
            nc.tensor.matmul(v_ps, wvb_sb[:, ko, :], x_sb[:, ko, cs],
                             start=(ko == 0), stop=(ko == KT - 1))
        nc.vector.tensor_add(v_sb[:, cs], v_ps, vadd_sb[:, cs])

    decay_t = decay_c.broadcast_to([D_STATE, TLOC])

    l_sb = vadd_sb  # vadd is dead once v is finalized
    nc.vector.tensor_tensor_scan(l_sb, decay_t, v_sb, 0.0,
                                 mybir.AluOpType.mult, mybir.AluOpType.add)

    cc_in = drm.tile([D_STATE, 1], F32)
    cc_out = drm.tile([D_STATE * N_CORES, 1], F32, addr_space="Shared")
    nc.sync.dma_start(out=cc_in[:], in_=l_sb[:, TLOC - 1 : TLOC])
    nc.gpsimd.collective_compute(
        "AllGather", mybir.AluOpType.bypass,
        replica_groups=[list(range(N_CORES))],
        ins=[cc_in[:]], outs=[cc_out[:]],
    )
    lam_all = sb.tile([D_STATE, N_CORES], F32)
    nc.sync.dma_start(out=lam_all,
                      in_=cc_out.rearrange("(j d) one -> d (j one)", d=D_STATE))
    sig_scr = sb.tile([D_STATE, N_CORES], F32)
    sigma = sb.tile([D_STATE, 1], F32)
    nc.vector.scalar_tensor_tensor(
        out=sig_scr, in0=lam_all, scalar=1.0, in1=gct_sb,
        op0=mybir.AluOpType.mult, op1=mybir.AluOpType.mult, accum_out=sigma)
    nc.vector.tensor_tensor_scan(s_sb, decay_t, v_sb, sigma,
                                 mybir.AluOpType.mult, mybir.AluOpType.add)

    # ---- in_proj: fp8 DR, z/u feature pairs, 2-bank PSUM drains ------
    u_sb = sb.tile([128, CT, TLOC + 3], BF16)      # halo(3) + tokens
    nc.sync.dma_start(out=u_sb[:, :, 0:3], in_=uhalo[:])
    sigz_sb = sb.tile([128, CT, TLOC], BF16)
    for f in f_order:
        if f in win_pre:
            wt = win_pre[f]
        else:
            wt = sb2.tile([128, DRK, 2, 128], FP8, name="win", bufs=4)
            nc.sync.dma_start(out=wt, in_=win8[f])
        p_t = ps.tile([128, TLOC], F32, tag="mm", name=f"ip{f}")
        for n in range(NCH):
            cs = slice(n * 512, (n + 1) * 512)
            for kp in range(DRK):
                nc.tensor.matmul(p_t[:, cs], wt[:, kp], x8[:, kp, :, cs],
                                 start=(kp == 0), stop=(kp == DRK - 1),
                                 perf_mode=DR)
        if f < CT:   # x-part -> u, scaled drain (bias folded into conv bias)
            nc.scalar.activation(
                out=u_sb[:, f, 3 : 3 + TLOC], in_=p_t,
                func=AF.Copy, bias=0.0, scale=wsc_sb[:, f : f + 1])
        else:        # z -> sigmoid(scale*z + bias)
            c = f - CT
            nc.scalar.activation(
                out=sigz_sb[:, c, :], in_=p_t, func=AF.Sigmoid,
                bias=biasz_sb[:, c : c + 1], scale=wsc_sb[:, f : f + 1])

    # ---- conv (DVE 4x bf16 MACs) + silu ------------------------------
    for c in range(CT):
        acc = sb2.tile([128, TLOC], BF16, name="cacc", bufs=2)
        nc.vector.tensor_scalar_mul(
            out=acc, in0=u_sb[:, c, 0:TLOC], scalar1=convw_sb[:, c, 0:1])
        for tap in range(1, D_CONV):
            nc.vector.scalar_tensor_tensor(
                out=acc, in0=u_sb[:, c, tap : tap + TLOC],
                scalar=convw_sb[:, c, tap : tap + 1], in1=acc,
                op0=mybir.AluOpType.mult, op1=mybir.AluOpType.add)
        nc.scalar.activation(
            out=u_sb[:, c, 3 : 3 + TLOC], in_=acc,
            func=AF.Silu, bias=convb_sb[:, c : c + 1], scale=1.0)

    # ---- sC + gating, chunk-split; out_proj zipped in -----------------
    cmt_sb = sb.tile([D_STATE, D_INNER], F32R)
    nc.sync.dma_start(out=cmt_sb, in_=cmt[:])
    y8 = sb.tile([128, CT, TLOC], FP8)

    def emit_sc(c, n):
        cs = slice(n * 512, (n + 1) * 512)
        sc_ps = pss.tile([128, 512], F32, tag="sm", name=f"sc{c}_{n}")
        nc.tensor.matmul(sc_ps, cmt_sb[:, c * 128 : (c + 1) * 128],
                         s_sb[:, cs], start=True, stop=False)
        nc.tensor.matmul(sc_ps, ident,
                         u_sb[:, c, 3 + n * 512 : 3 + (n + 1) * 512],
                         start=False, stop=True)
        nc.vector.tensor_mul(y8[:, c, cs], sc_ps, sigz_sb[:, c, cs])

    wo_tiles = {}
    for m in range(MT):
        w = sb2.tile([128, OKT, 2, 128], FP8, name="wo", bufs=8)
        nc.sync.dma_start(out=w, in_=wot8[m])
        wo_tiles[m] = w

    out_ps = {}

    def emit_out_k(m, n, c):
        # k-tile c of out_proj group (m, chunk n); hi/lo pair vs same y8 tile
        cs = slice(n * 512, (n + 1) * 512)
        if (m, n) not in out_ps:
            out_ps[(m, n)] = ps.tile([128, 512], F32, tag="om",
                                     name=f"op{m}_{n}", bufs=2)
        o_t = out_ps[(m, n)]
        rhs = y8[:, c : c + 1, cs].broadcast_to([128, 2, 512])
        nc.tensor.matmul(o_t, wo_tiles[m][:, c], rhs,
                         start=(c == 0), stop=(c == OKT - 1), perf_mode=DR)

    def drain_out(m, n):
        cs = slice(n * 512, (n + 1) * 512)
        r_sb = sb2.tile([128, 512], BF16, name="r_sb", bufs=2)
        nc.scalar.activation(out=r_sb, in_=out_ps[(m, n)], func=AF.Copy,
                             bias=0.0, scale=wosc_sb[:, m : m + 1])
        nc.sync.dma_start(out=res[m * 128 : (m + 1) * 128, cs], in_=r_sb)

    # chunk 0 sC/gating
    for c in range(CT):
        emit_sc(c, 0)
    # zip: chunk-1 sC with the first chunk-0 out_proj wave (m0, m1)
    for c in range(CT):
        emit_sc(c, 1)
        emit_out_k(0, 0, c)
        emit_out_k(1, 0, c)
    drain_out(0, 0)
    drain_out(1, 0)
    # remaining waves of two groups ("om" has 2 slots)
    waves = [((2, 0), (3, 0)), ((4, 0), (5, 0)), ((6, 0), (7, 0)),
             ((0, 1), (1, 1)), ((2, 1), (3, 1)), ((4, 1), (5, 1)),
             ((6, 1), (7, 1))]
    for (ma, na), (mb, nb) in waves:
        for c in range(CT):
            emit_out_k(ma, na, c)
            emit_out_k(mb, nb, c)
        drain_out(ma, na)
        drain_out(mb, nb)


# ---------------------------------------------------------------------
# host side
# ---------------------------------------------------------------------

def _standardize(x):
    mu = x.mean(-1, keepdims=True)
    var = ((x - mu) ** 2).mean(-1, keepdims=True)
    return ((x - mu) / np.sqrt(var + LN_EPS)).astype(np.float32)


def _qrow8(W, cap=224.0):
    """Per-row absmax e4m3 quantization. Returns (q, scale)."""
    s = np.abs(W).max(axis=1, keepdims=True) / cap
    s = np.maximum(s, 1e-30)
    return (W / s).astype(E4), s[:, 0].astype(np.float32)


def host_prepare(inputs):
    x = np.ascontiguousarray(np.asarray(inputs["x"], np.float32))
    g = np.asarray(inputs["ln_gamma"], np.float32)
    beta = np.asarray(inputs["ln_beta"], np.float32)
    W_in = np.asarray(inputs["W_in"], np.float32)
    conv_w = np.asarray(inputs["conv_w"], np.float32)[:, 0, :]
    conv_b = np.asarray(inputs["conv_b"], np.float32)
    W_out = np.asarray(inputs["W_out"], np.float32)
    A = np.asarray(inputs["A"], np.float32)
    Bm = np.asarray(inputs["Bm"], np.float32)
    Cm = np.asarray(inputs["Cm"], np.float32)

    Wg = W_in * g[None, :]
    b_in = W_in @ beta
    bias_u = b_in[:D_INNER]
    bias_z = b_in[D_INNER:]
    W1g = Wg[:D_INNER]

    # in_proj fp8 packing: per-row scale, DR pair layout
    W8, sW = _qrow8(Wg)
    win8 = np.empty((FT, 128, DRK, 2, 128), dtype=E4)
    for f in range(FT):
        blk = W8[f * 128 : (f + 1) * 128]          # [M=128, K=1024]
        win8[f] = blk.T.reshape(DRK, 2, 128, 128).transpose(2, 0, 1, 3)
    wsc_p = np.ascontiguousarray(sW.reshape(FT, 128).T)

    # out_proj fp8 hi/lo packing with shared per-row scale
    sO = np.abs(W_out).max(axis=1, keepdims=True) / 224.0
    sO = np.maximum(sO, 1e-30)
    Wo = W_out / sO
    Whi = Wo.astype(E4)
    Wlo = (Wo - Whi.astype(np.float32)).astype(E4)
    wot8 = np.empty((MT, 128, OKT, 2, 128), dtype=E4)
    for m in range(MT):
        hi = Whi[m * 128 : (m + 1) * 128].astype(E4)     # [128, 2048]
        lo = Wlo[m * 128 : (m + 1) * 128].astype(E4)
        stacked = np.stack([hi.T, lo.T], axis=1)          # [2048, 2, 128]
        wot8[m] = stacked.reshape(OKT, 128, 2, 128).transpose(1, 0, 2, 3)
    wosc_p = np.ascontiguousarray(sO[:, 0].reshape(MT, 128).T)

    Wvb0 = (Bm @ W_in[:D_INNER]) * g[None, :]
    bias_v0 = Bm @ W_in[:D_INNER] @ beta

    fallback = False
    lamc, V = np.linalg.eig(A.astype(np.float64))
    if np.abs(lamc.imag).max() > 1e-9 or np.linalg.cond(V) > 1e3:
        fallback = True
    if fallback:
        lam = np.zeros(D_STATE, np.float32)
        Wvb = np.zeros_like(Wvb0)
        Cmt = Cm.astype(np.float32)
        xn = _standardize(x.reshape(-1, D_MODEL)).reshape(x.shape) * g + beta
        v = xn.astype(np.float32) @ (Bm @ W_in[:D_INNER]).T
        sT = np.zeros((B, L, D_STATE), np.float32)
        for b_ in range(B):
            cur = np.zeros(D_STATE, np.float64)
            Ad = A.astype(np.float64)
            for tt in range(L):
                cur = Ad @ cur + v[b_, tt]
                sT[b_, tt] = cur
        sT = np.nan_to_num(sT, posinf=3e38, neginf=-3e38)
    else:
        lam = lamc.real
        Vr = V.real
        Vi = np.linalg.inv(Vr)
        Wvb = (Vi @ Wvb0).astype(np.float32)
        bias_vt = (Vi @ bias_v0).astype(np.float32)
        Cmt = (Vr.T @ Cm).astype(np.float32)

    wvbt = np.ascontiguousarray(
        Wvb.reshape(D_STATE, KT, 128).transpose(2, 1, 0)).astype(BF) \
        if not fallback else np.zeros((128, KT, D_STATE), BF)

    convw_p = np.ascontiguousarray(conv_w.reshape(CT, 128, D_CONV).transpose(1, 0, 2))
    # u is produced biasless on device; fold bias_u through the conv taps
    convb_f = conv_b + bias_u * conv_w.sum(axis=1)
    convb_p = np.ascontiguousarray(convb_f.reshape(CT, 128).T)
    biasz_p = np.ascontiguousarray(bias_z.reshape(CT, 128).T)
    decay_p = lam.astype(np.float32).reshape(D_STATE, 1)

    in_maps = []
    for c in range(N_CORES):
        b_, k = c // 4, c % 4
        xs = x[b_, k * TLOC : (k + 1) * TLOC]            # (1024, 1024)
        xTc = np.ascontiguousarray(xs.T).astype(BF)

        if k == 0:
            uh = np.zeros((D_INNER, 3), np.float32)
        else:
            xh = x[b_, k * TLOC - 3 : k * TLOC]
            uh = (_standardize(xh) @ W1g.T).T  # biasless; bias folded into conv_b
        uh_p = np.ascontiguousarray(
            uh.reshape(CT, 128, 3).transpose(1, 0, 2)).astype(BF)

        if fallback:
            va = np.ascontiguousarray(sT[b_, k * TLOC : (k + 1) * TLOC].T)
            G = np.zeros((N_CORES, D_STATE), np.float32)
        else:
            va = np.broadcast_to(bias_vt[:, None], (D_STATE, TLOC)).copy()
            G = np.zeros((N_CORES, D_STATE), np.float32)
            for j in range(N_CORES):
                bj, kj = j // 4, j % 4
                if bj == b_ and kj < k:
                    G[j] = lam ** (TLOC * (k - kj))
        in_maps.append(dict(
            xT=xTc, win8=win8, wsc=wsc_p, wvbt=wvbt,
            cmt=Cmt.astype(np.float32), wot8=wot8, wosc=wosc_p,
            convw=convw_p, convb=convb_p, biasz=biasz_p, uhalo=uh_p,
            vadd=va.astype(np.float32), decay=decay_p,
            gct=np.ascontiguousarray(G.T),
        ))
    return in_maps, x


def get_nc():
    global _NC_CACHE
    if _NC_CACHE is None:
        _NC_CACHE = build_graph()
    return _NC_CACHE


def kernel(**inputs):
    global LAST_RESULT
    nc = get_nc()
    in_maps, x = host_prepare(inputs)
    trace = bool(os.environ.get("BASS_TRACE"))
    r = run_bass_kernel_spmd(nc, in_maps, core_ids=list(range(N_CORES)),
                             trace=trace)
    LAST_RESULT = r
    out = np.empty((B, L, D_MODEL), np.float32)
    for c in range(N_CORES):
        b_, k = c // 4, c % 4
        resT = r.results[c]["res"].astype(np.float32)    # (d_model, tok) bf16
        out[b_, k * TLOC : (k + 1) * TLOC] = (
            x[b_, k * TLOC : (k + 1) * TLOC] + resT.T)
    return out


# revision 5
# speedup vs baseline: 1.6997x; 1.2702x over previous
"""AdaptiveMambaBlock on 8 TRN2 NeuronCores (Bass/Tile, SPMD) — fp8 DoubleRow.

Sharding: 8-way over tokens. Core c handles batch c//4, token range
[(c%4)*1024, +1024). Feature-major layout; host pre-packs weights.

v3: all heavy GEMM-like work runs on the PE as fp8e4m3 DoubleRow matmuls
(K=256 per instruction at 0.5 cyc/row):
  - in_proj: both operands fp8 (per-row weight scales folded into drains)
  - out_proj: weight split hi+lo e4m3 sharing one row scale, paired in one
    DR instruction against a stride-0-broadcast y8 k-tile
  - depthwise conv: diagonal fp8 tap matrices; DR pairs read a 1-token-
    shifted SBUF duplicate of u8 so both windows slice with clean strides
  - the sC "+xc" add: identity(fp8)+zero DR against the silu output
The d_state recurrence (v, scan, collective stitch, cmt@s) stays bf16/f32
since the cumsum amplifies quantization error.

Elementwise: normalize/squares in bf16 on DVE (2x modes), psum drains
grouped per activation function to avoid act-table reloads (Copy lives in
every table; Sigmoid/Silu each get one load), gating on DVE, xhat fp8
quantization split across Pool/DVE/Act.

Scan: s^c_t = A s^c_{t-1} + v^c_t with v from the normalized input
(v = xhat @ (Bm@W1*gamma).T), A diagonalized on the host; cross-core state
stitched with a 64B-per-core AllGather of the local scan tails.
"""

import os
import numpy as np
import ml_dtypes

import concourse.bass as bass
import concourse.tile as tile
from concourse import bacc
from concourse import mybir
from concourse.bass_utils import run_bass_kernel_spmd
from concourse.masks import make_identity

F32 = mybir.dt.float32
F32R = mybir.dt.float32r
BF16 = mybir.dt.bfloat16
FP8 = mybir.dt.float8e4
E4 = ml_dtypes.float8_e4m3
BF = ml_dtypes.bfloat16
DRM = mybir.MatmulPerfMode.DoubleRow

D_MODEL, D_STATE, D_CONV = 1024, 16, 4
D_INNER = 2048
B, L = 2, 4096
LN_EPS = 1e-5
N_CORES = 8
TLOC = 1024              # tokens per core
KT = D_MODEL // 128      # 8 K tiles over d_model
DRK = KT // 2            # 4 DoubleRow K pairs for in_proj
FT = 2 * D_INNER // 128  # 32 feature tiles (x-part 0..15, z 16..31)
CT = D_INNER // 128      # 16 channel tiles
MT = D_MODEL // 128      # 8 output (d_model) tiles
OKT = D_INNER // 128     # 16 out_proj K tiles
NCH = TLOC // 512        # 2 token chunks of 512
UW = TLOC + 3            # u8 row width (halo 3 + tokens)

_NC_CACHE = None
LAST_RESULT = None


def build_graph():
    nc = bacc.Bacc(num_devices=N_CORES)

    xT = nc.declare_dram_parameter("xT", [D_MODEL, TLOC], BF16, isOutput=False)
    win8 = nc.declare_dram_parameter("win8", [FT, 128, DRK, 2, 128], FP8, isOutput=False)
    wsc = nc.declare_dram_parameter("wsc", [128, FT], F32, isOutput=False)
    wvbt = nc.declare_dram_parameter("wvbt", [128, KT, D_STATE], BF16, isOutput=False)
    cmt = nc.declare_dram_parameter("cmt", [D_STATE, D_INNER], F32R, isOutput=False)
    wot8 = nc.declare_dram_parameter("wot8", [MT, 128, OKT, 2, 128], FP8, isOutput=False)
    wosc = nc.declare_dram_parameter("wosc", [128, MT], F32, isOutput=False)
    convd8 = nc.declare_dram_parameter("convd8", [128, CT, 2, 2, 128], FP8, isOutput=False)
    convsc = nc.declare_dram_parameter("convsc", [128, CT], F32, isOutput=False)
    convb = nc.declare_dram_parameter("convb", [128, CT], F32, isOutput=False)
    biasz = nc.declare_dram_parameter("biasz", [128, CT], F32, isOutput=False)
    uhalo = nc.declare_dram_parameter("uhalo", [128, CT, 3], FP8, isOutput=False)
    vadd = nc.declare_dram_parameter("vadd", [D_STATE, TLOC], F32, isOutput=False)
    decay = nc.declare_dram_parameter("decay", [D_STATE, 1], F32, isOutput=False)
    gct = nc.declare_dram_parameter("gct", [D_STATE, N_CORES], F32, isOutput=False)
    res = nc.declare_dram_parameter("res", [D_MODEL, TLOC], BF16, isOutput=True)

    with tile.TileContext(nc) as tc:
        with (
            tc.tile_pool(name="sb", bufs=1) as sb,
            tc.tile_pool(name="sb2", bufs=2) as sb2,
            tc.tile_pool(name="ps", bufs=2, space="PSUM") as ps,
            tc.tile_pool(name="pss", bufs=2, space="PSUM") as pss,
            tc.tile_pool(name="dr", bufs=1, space="DRAM") as drm,
        ):
            with nc.allow_low_precision(reason="fp8/bf16 matmul pipeline"):
                _emit(nc, tc, sb, sb2, ps, pss, drm, locals())
    nc.compile()
    return nc


def _emit(nc, tc, sb, sb2, ps, pss, drm, t):
    xT, win8, wsc, wvbt, cmt, wot8, wosc = (
        t["xT"], t["win8"], t["wsc"], t["wvbt"], t["cmt"], t["wot8"], t["wosc"])
    convd8, convsc, convb, biasz = t["convd8"], t["convsc"], t["convb"], t["biasz"]
    uhalo, vadd, decay, gct, res = t["uhalo"], t["vadd"], t["decay"], t["gct"], t["res"]

    AF = mybir.ActivationFunctionType

    # ---- x load first (per half, per K tile): gates the stats chain.
    x_sb = sb.tile([128, KT, TLOC], BF16)
    xTr = xT.rearrange("(ko ki) t -> ki ko t", ki=128)
    win_pre = {}
    for ko in range(KT):
        nc.sync.dma_start(out=x_sb[:, ko, 0:512], in_=xTr[:, ko, 0:512])
    for f in (0, 1):
        w = sb2.tile([128, DRK, 2, 128], FP8, name="win", bufs=4)
        nc.sync.dma_start(out=w, in_=win8[f])
        win_pre[f] = w
    for ko in range(KT):
        nc.sync.dma_start(out=x_sb[:, ko, 512:1024], in_=xTr[:, ko, 512:1024])

    # ---- constants / small loads -------------------------------------
    ones_k0 = sb.tile([128, 1], F32)
    nc.vector.memset(ones_k0, 1.0 / D_MODEL)
    ones_k = sb.tile([128, 1], BF16)         # 1/D_MODEL, stats lhsT
    nc.vector.tensor_copy(ones_k, ones_k0)
    ones_b0 = sb.tile([1, 128], F32)
    nc.vector.memset(ones_b0, 1.0)
    ones_b = sb.tile([1, 128], BF16)         # broadcast lhsT (K=1)
    nc.vector.tensor_copy(ones_b, ones_b0)
    rr = sb.tile([1, 1024], BF16)            # bf16 staging rows for bcast rhs
    eps_t = sb.tile([1, 1], F32)
    nc.vector.memset(eps_t, LN_EPS)
    id2 = sb.tile([128, 2, 128], FP8)        # identity | zeros, DR pair
    nc.gpsimd.memset(id2, 0.0)
    make_identity(nc, id2[:, 0, :], nomemset=True)
    convd_sb = sb.tile([128, CT, 2, 2, 128], FP8)
    nc.sync.dma_start(out=convd_sb, in_=convd8[:])
    convsc_sb = sb.tile([128, CT], F32)
    nc.sync.dma_start(out=convsc_sb, in_=convsc[:])
    convb_sb = sb.tile([128, CT], F32)
    nc.sync.dma_start(out=convb_sb, in_=convb[:])
    biasz_sb = sb.tile([128, CT], F32)
    nc.sync.dma_start(out=biasz_sb, in_=biasz[:])
    wsc_sb = sb.tile([128, FT], F32)
    nc.sync.dma_start(out=wsc_sb, in_=wsc[:])
    wosc_sb = sb.tile([128, MT], F32)
    nc.sync.dma_start(out=wosc_sb, in_=wosc[:])

    rows = sb.tile([1, 3 * 512], F32)        # per-half: mu | var | scratch
    states = sb.tile([D_STATE, 2 * TLOC], F32)
    s_sb = sb.tile([D_STATE, TLOC], F32R)    # scan #2 output (sC rhs)
    vadd_sb = states[:, TLOC : 2 * TLOC]
    nc.sync.dma_start(out=vadd_sb, in_=vadd[:])
    decay_c = sb.tile([D_STATE, 1], F32)
    nc.sync.dma_start(out=decay_c, in_=decay[:])
    gct_sb = sb.tile([D_STATE, N_CORES], F32)
    nc.sync.dma_start(out=gct_sb, in_=gct[:])
    wvb_sb = sb.tile([128, KT, D_STATE], BF16)
    nc.sync.dma_start(out=wvb_sb, in_=wvbt[:])

    x8 = sb.tile([128, DRK, 2, TLOC], FP8)   # quantized xhat, DR rhs layout
    rb_sb = sb.tile([128, TLOC], BF16)
    murb_sb = sb.tile([128, TLOC], BF16)
    # u8d: [copy0 | copy1], copy1 = copy0 shifted left 1 token (SBUF DMA dup)
    u8d = sb.tile([128, 2, CT, UW], FP8)
    nc.sync.dma_start(out=u8d[:, 0, :, 0:3], in_=uhalo[:])
    xc8 = sb.tile([128, CT, TLOC], FP8)      # silu(conv) output
    sigz_sb = sb.tile([128, CT, TLOC], BF16)

    mu_row = rows[:, 0:512]
    var_row = rows[:, 512:1024]
    mu2_row = rows[:, 1024:1536]

    # ---- layernorm stats + normalize + fp8 quantize (per half) -------
    def stats_half(n):
        cs = slice(n * 512, (n + 1) * 512)
        mu_ps = pss.tile([1, 512], F32, tag="sm", name=f"mu_ps{n}")
        sq_ps = pss.tile([1, 512], F32, tag="sm", name=f"sq_ps{n}")
        for ko in range(KT):
            sq_scr = sb2.tile([128, 512], BF16, name="sq_scr", bufs=2)
            if n == 0:
                nc.vector.tensor_mul(sq_scr, x_sb[:, ko, cs], x_sb[:, ko, cs])
            else:
                nc.scalar.square(sq_scr, x_sb[:, ko, cs])
            nc.tensor.matmul(mu_ps, ones_k, x_sb[:, ko, cs],
                             start=(ko == 0), stop=(ko == KT - 1))
            nc.tensor.matmul(sq_ps, ones_k, sq_scr,
                             start=(ko == 0), stop=(ko == KT - 1))
        nc.vector.tensor_copy(mu_row, mu_ps)
        nc.vector.tensor_mul(mu2_row, mu_row, mu_ps)
        nc.vector.tensor_sub(var_row, sq_ps, mu2_row)
        rr_r, rr_mur = rr[:, 0:512], rr[:, 512:1024]
        nc.scalar.activation(rr_r, var_row, AF.Abs_reciprocal_sqrt,
                             bias=eps_t, scale=1.0)
        nc.vector.tensor_mul(rr_mur, mu_row, rr_r)
        for srow, dst in ((rr_r, rb_sb), (rr_mur, murb_sb)):
            b_ps = pss.tile([128, 512], F32, tag="sm", name="b_ps")
            nc.tensor.matmul(b_ps, ones_b, srow, start=True, stop=True)
            nc.vector.tensor_copy(dst[:, cs], b_ps)

    def norm_quant_half(n):
        cs = slice(n * 512, (n + 1) * 512)
        for ko in range(KT):
            nc.vector.tensor_mul(x_sb[:, ko, cs], x_sb[:, ko, cs], rb_sb[:, cs])
            nc.vector.tensor_sub(x_sb[:, ko, cs], x_sb[:, ko, cs], murb_sb[:, cs])
        for ko in range(KT):
            dst = x8[:, ko // 2, ko % 2, cs]
            if n == 0:
                if ko < 4:
                    nc.gpsimd.tensor_copy(dst, x_sb[:, ko, cs])
                else:
                    nc.vector.tensor_copy(dst, x_sb[:, ko, cs])
            else:
                nc.scalar.copy(dst, x_sb[:, ko, cs])

    def v_half(n):
        cs = slice(n * 512, (n + 1) * 512)
        v_ps = pss.tile([D_STATE, 512], F32, tag="sm", name="v_ps")
        for ko in range(KT):
            nc.tensor.matmul(v_ps, wvb_sb[:, ko, :], x_sb[:, ko, cs],
                             start=(ko == 0), stop=(ko == KT - 1))
        nc.vector.tensor_add(states[:, cs], v_ps, vadd_sb[:, cs])

    stats_half(0)
    norm_quant_half(0)
    stats_half(1)
    norm_quant_half(1)
    v_half(0)

    # ---- in_proj f-loop (fp8 DR), u8/sigz drains ---------------------
    def in_proj_f(f):
        if f in win_pre:
            wt = win_pre[f]
        else:
            wt = sb2.tile([128, DRK, 2, 128], FP8, name="win", bufs=4)
            nc.sync.dma_start(out=wt, in_=win8[f])
        p_t = ps.tile([128, TLOC], F32, tag="mm", name=f"ip{f}")
        for n in range(NCH):
            cs = slice(n * 512, (n + 1) * 512)
            for kp in range(DRK):
                nc.tensor.matmul(p_t[:, cs], wt[:, kp], x8[:, kp, :, cs],
                                 start=(kp == 0), stop=(kp == DRK - 1),
                                 perf_mode=DRM)
        if f < CT:   # x-part -> u8 (scaled fp8 drain on DVE)
            nc.vector.tensor_scalar_mul(
                out=u8d[:, 0, f, 3:3 + TLOC], in0=p_t,
                scalar1=wsc_sb[:, f : f + 1])
            # shifted duplicate for the conv DR tap pairs
            nc.sync.dma_start(out=u8d[:, 1, f, 0 : UW - 1],
                              in_=u8d[:, 0, f, 1:UW])
        else:        # z -> sigmoid(scale*z + bias)
            c = f - CT
            nc.scalar.activation(
                out=sigz_sb[:, c, :], in_=p_t, func=AF.Sigmoid,
                bias=biasz_sb[:, c : c + 1], scale=wsc_sb[:, f : f + 1])

    in_proj_f(0)
    v_half(1)

    decay_t = decay_c.broadcast_to([D_STATE, TLOC])
    v_sb = states[:, 0:TLOC]
    l_sb = vadd_sb  # vadd is dead once v is finalized
    nc.vector.tensor_tensor_scan(l_sb, decay_t, v_sb, 0.0,
                                 mybir.AluOpType.mult, mybir.AluOpType.add)

    cc_in = drm.tile([D_STATE, 1], F32)
    cc_out = drm.tile([D_STATE * N_CORES, 1], F32, addr_space="Shared")
    nc.sync.dma_start(out=cc_in[:], in_=l_sb[:, TLOC - 1 : TLOC])
    nc.gpsimd.collective_compute(
        "AllGather", mybir.AluOpType.bypass,
        replica_groups=[list(range(N_CORES))],
        ins=[cc_in[:]], outs=[cc_out[:]],
    )
    lam_all = sb.tile([D_STATE, N_CORES], F32)
    nc.sync.dma_start(out=lam_all,
                      in_=cc_out.rearrange("(j d) one -> d (j one)", d=D_STATE))
    sig_scr = sb.tile([D_STATE, N_CORES], F32)
    sigma = sb.tile([D_STATE, 1], F32)
    nc.vector.scalar_tensor_tensor(
        out=sig_scr, in0=lam_all, scalar=1.0, in1=gct_sb,
        op0=mybir.AluOpType.mult, op1=mybir.AluOpType.mult, accum_out=sigma)
    nc.vector.tensor_tensor_scan(s_sb, decay_t, v_sb, sigma,
                                 mybir.AluOpType.mult, mybir.AluOpType.add)

    for f in range(1, FT):
        in_proj_f(f)

    # ---- conv on PE (fp8 DR diag taps) + silu -> xc8 ------------------
    for c in range(CT):
        cp = ps.tile([128, TLOC], F32, tag="mm", name=f"cv{c}")
        for n in range(NCH):
            cs = slice(n * 512, (n + 1) * 512)
            for p in range(2):
                a = 2 * p + n * 512
                rhs = u8d[:, :, c, a : a + 512]     # [128, 2, 512]
                nc.tensor.matmul(cp[:, cs], convd_sb[:, c, p], rhs,
                                 start=(p == 0), stop=(p == 1),
                                 perf_mode=DRM)
        nc.scalar.activation(
            out=xc8[:, c, :], in_=cp, func=AF.Silu,
            bias=convb_sb[:, c : c + 1], scale=convsc_sb[:, c : c + 1])

    # ---- sC + gating, chunk-split; out_proj zipped in -----------------
    cmt_sb = sb.tile([D_STATE, D_INNER], F32R)
    nc.sync.dma_start(out=cmt_sb, in_=cmt[:])
    y8 = sb.tile([128, CT, TLOC], FP8)

    def emit_sc(c, n):
        cs = slice(n * 512, (n + 1) * 512)
        sc_ps = pss.tile([128, 512], F32, tag="sm", name=f"sc{c}_{n}")
        nc.tensor.matmul(sc_ps, cmt_sb[:, c * 128 : (c + 1) * 128],
                         s_sb[:, cs], start=True, stop=False)
        rhs = xc8[:, c : c + 1, cs].broadcast_to([128, 2, 512])
        nc.tensor.matmul(sc_ps, id2, rhs, start=False, stop=True,
                         perf_mode=DRM)
        nc.vector.tensor_mul(y8[:, c, cs], sc_ps, sigz_sb[:, c, cs])

    wo_tiles = {}
    for m in range(MT):
        w = sb2.tile([128, OKT, 2, 128], FP8, name="wo", bufs=8)
        nc.sync.dma_start(out=w, in_=wot8[m])
        wo_tiles[m] = w

    out_ps = {}

    def emit_out_k(m, n, c):
        # k-tile c of out_proj group (m, chunk n); hi/lo pair vs same y8 tile
        cs = slice(n * 512, (n + 1) * 512)
        if (m, n) not in out_ps:
            out_ps[(m, n)] = ps.tile([128, 512], F32, tag="om",
                                     name=f"op{m}_{n}", bufs=2)
        o_t = out_ps[(m, n)]
        rhs = y8[:, c : c + 1, cs].broadcast_to([128, 2, 512])
        nc.tensor.matmul(o_t, wo_tiles[m][:, c], rhs,
                         start=(c == 0), stop=(c == OKT - 1), perf_mode=DRM)

    def drain_out(m, n):
        cs = slice(n * 512, (n + 1) * 512)
        r_sb = sb2.tile([128, 512], BF16, name="r_sb", bufs=2)
        nc.scalar.activation(out=r_sb, in_=out_ps[(m, n)], func=AF.Copy,
                             bias=0.0, scale=wosc_sb[:, m : m + 1])
        nc.sync.dma_start(out=res[m * 128 : (m + 1) * 128, cs], in_=r_sb)

    # chunk 0 sC/gating
    for c in range(CT):
        emit_sc(c, 0)
    # zip: chunk-1 sC with the first chunk-0 out_proj wave (m0, m1)
    for c in range(CT):
        emit_sc(c, 1)
        emit_out_k(0, 0, c)
        emit_out_k(1, 0, c)
    drain_out(0, 0)
    drain_out(1, 0)
    waves = [((2, 0), (3, 0)), ((4, 0), (5, 0)), ((6, 0), (7, 0)),
             ((0, 1), (1, 1)), ((2, 1), (3, 1)), ((4, 1), (5, 1)),
             ((6, 1), (7, 1))]
    for (ma, na), (mb, nb) in waves:
        for c in range(CT):
            emit_out_k(ma, na, c)
            emit_out_k(mb, nb, c)
        drain_out(ma, na)
        drain_out(mb, nb)


# ---------------------------------------------------------------------
# host side
# ---------------------------------------------------------------------

def _standardize(x):
    mu = x.mean(-1, keepdims=True)
    var = ((x - mu) ** 2).mean(-1, keepdims=True)
    return ((x - mu) / np.sqrt(var + LN_EPS)).astype(np.float32)


def host_prepare(inputs):
    x = np.ascontiguousarray(np.asarray(inputs["x"], np.float32))
    g = np.asarray(inputs["ln_gamma"], np.float32)
    beta = np.asarray(inputs["ln_beta"], np.float32)
    W_in = np.asarray(inputs["W_in"], np.float32)
    conv_w = np.asarray(inputs["conv_w"], np.float32)[:, 0, :]
    conv_b = np.asarray(inputs["conv_b"], np.float32)
    W_out = np.asarray(inputs["W_out"], np.float32)
    A = np.asarray(inputs["A"], np.float32)
    Bm = np.asarray(inputs["Bm"], np.float32)
    Cm = np.asarray(inputs["Cm"], np.float32)

    Wg = W_in * g[None, :]
    b_in = W_in @ beta
    bias_u = b_in[:D_INNER]
    bias_z = b_in[D_INNER:]
    W1g = Wg[:D_INNER]

    # in_proj fp8 packing: per-row scale, DR pair layout
    sW = np.abs(Wg).max(axis=1, keepdims=True) / 224.0
    sW = np.maximum(sW, 1e-30)
    W8 = (Wg / sW).astype(E4)
    win8 = np.empty((FT, 128, DRK, 2, 128), dtype=E4)
    for f in range(FT):
        blk = W8[f * 128 : (f + 1) * 128]          # [M=128, K=1024]
        win8[f] = blk.T.reshape(DRK, 2, 128, 128).transpose(2, 0, 1, 3)
    wsc_p = np.ascontiguousarray(sW[:, 0].reshape(FT, 128).T)

    # out_proj fp8 hi/lo packing with shared per-row scale
    sO = np.abs(W_out).max(axis=1, keepdims=True) / 224.0
    sO = np.maximum(sO, 1e-30)
    Wo = W_out / sO
    Whi = Wo.astype(E4)
    Wlo = (Wo - Whi.astype(np.float32)).astype(E4)
    wot8 = np.empty((MT, 128, OKT, 2, 128), dtype=E4)
    for m in range(MT):
        hi = Whi[m * 128 : (m + 1) * 128]
        lo = Wlo[m * 128 : (m + 1) * 128]
        stacked = np.stack([hi.T, lo.T], axis=1)          # [2048, 2, 128]
        wot8[m] = stacked.reshape(OKT, 128, 2, 128).transpose(1, 0, 2, 3)
    wosc_p = np.ascontiguousarray(sO[:, 0].reshape(MT, 128).T)

    # depthwise conv: per-channel scaled e4m3 taps as diagonal DR pairs
    scw = np.abs(conv_w).max(axis=1) / 224.0
    scw = np.maximum(scw, 1e-30)
    w8t = (conv_w / scw[:, None]).astype(E4)              # [D_INNER, 4]
    convd8 = np.zeros((128, CT, 2, 2, 128), dtype=E4)
    mm = np.arange(128)
    for c in range(CT):
        for p in range(2):
            for sub in range(2):
                convd8[mm, c, p, sub, mm] = w8t[c * 128 + mm, 2 * p + sub]
    convsc_p = np.ascontiguousarray(scw.reshape(CT, 128).T)
    # device-effective taps for the bias fold (u is biasless on device)
    w_eff = w8t.astype(np.float32) * scw[:, None]
    convb_f = conv_b + bias_u * w_eff.sum(axis=1)
    convb_p = np.ascontiguousarray(convb_f.reshape(CT, 128).T)
    biasz_p = np.ascontiguousarray(bias_z.reshape(CT, 128).T)

    Wvb0 = (Bm @ W_in[:D_INNER]) * g[None, :]
    bias_v0 = Bm @ W_in[:D_INNER] @ beta

    fallback = False
    lamc, V = np.linalg.eig(A.astype(np.float64))
    if np.abs(lamc.imag).max() > 1e-9 or np.linalg.cond(V) > 1e3:
        fallback = True
    if fallback:
        lam = np.zeros(D_STATE, np.float32)
        Wvb = np.zeros_like(Wvb0)
        Cmt = Cm.astype(np.float32)
        xn = _standardize(x.reshape(-1, D_MODEL)).reshape(x.shape) * g + beta
        v = xn.astype(np.float32) @ (Bm @ W_in[:D_INNER]).T
        sT = np.zeros((B, L, D_STATE), np.float32)
        for b_ in range(B):
            cur = np.zeros(D_STATE, np.float64)
            Ad = A.astype(np.float64)
            for tt in range(L):
                cur = Ad @ cur + v[b_, tt]
                sT[b_, tt] = cur
        sT = np.nan_to_num(sT, posinf=3e38, neginf=-3e38)
    else:
        lam = lamc.real
        Vr = V.real
        Vi = np.linalg.inv(Vr)
        Wvb = (Vi @ Wvb0).astype(np.float32)
        bias_vt = (Vi @ bias_v0).astype(np.float32)
        Cmt = (Vr.T @ Cm).astype(np.float32)

    wvbt = np.ascontiguousarray(
        Wvb.reshape(D_STATE, KT, 128).transpose(2, 1, 0)).astype(BF) \
        if not fallback else np.zeros((128, KT, D_STATE), BF)

    decay_p = lam.astype(np.float32).reshape(D_STATE, 1)

    in_maps = []
    for c in range(N_CORES):
        b_, k = c // 4, c % 4
        xs = x[b_, k * TLOC : (k + 1) * TLOC]            # (1024, 1024)
        xTc = np.ascontiguousarray(xs.T).astype(BF)

        if k == 0:
            uh = np.zeros((D_INNER, 3), np.float32)
        else:
            xh = x[b_, k * TLOC - 3 : k * TLOC]
            uh = (_standardize(xh) @ W1g.T).T  # biasless; bias folded into conv_b
        uh_p = np.ascontiguousarray(
            uh.reshape(CT, 128, 3).transpose(1, 0, 2)).astype(E4)

        if fallback:
            va = np.ascontiguousarray(sT[b_, k * TLOC : (k + 1) * TLOC].T)
            G = np.zeros((N_CORES, D_STATE), np.float32)
        else:
            va = np.broadcast_to(bias_vt[:, None], (D_STATE, TLOC)).copy()
            G = np.zeros((N_CORES, D_STATE), np.float32)
            for j in range(N_CORES):
                bj, kj = j // 4, j % 4
                if bj == b_ and kj < k:
                    G[j] = lam ** (TLOC * (k - kj))
        in_maps.append(dict(
            xT=xTc, win8=win8, wsc=wsc_p, wvbt=wvbt,
            cmt=Cmt.astype(np.float32), wot8=wot8, wosc=wosc_p,
            convd8=convd8, convsc=convsc_p, convb=convb_p,
            biasz=biasz_p, uhalo=uh_p,
            vadd=va.astype(np.float32), decay=decay_p,
            gct=np.ascontiguousarray(G.T),
        ))
    return in_maps, x


def get_nc():
    global _NC_CACHE
    if _NC_CACHE is None:
        _NC_CACHE = build_graph()
    return _NC_CACHE


def kernel(**inputs):
    global LAST_RESULT
    nc = get_nc()
    in_maps, x = host_prepare(inputs)
    trace = bool(os.environ.get("BASS_TRACE"))
    r = run_bass_kernel_spmd(nc, in_maps, core_ids=list(range(N_CORES)),
                             trace=trace)
    LAST_RESULT = r
    out = np.empty((B, L, D_MODEL), np.float32)
    for c in range(N_CORES):
        b_, k = c // 4, c % 4
        resT = r.results[c]["res"].astype(np.float32)    # (d_model, tok) bf16
        out[b_, k * TLOC : (k + 1) * TLOC] = (
            x[b_, k * TLOC : (k + 1) * TLOC] + resT.T)
    return out
